# revision 27
# baseline (speedup 1.0000x reference)
"""Window-routed sparse attention on 8 TRN2 NeuronCores.

Sharding: 64 windows x 8 cores = 8 windows/core (embarrassingly parallel).
Host precomputes the tiny routing path (region means, a_r [64,64]) and the
window-mixed q_m/k_m in fp32 numpy; each core runs the heavy windowed
attention relu(q_m k_m^T) v for its 8 windows.

Device-side layout: windows are processed in PAIRS so the 128x128 PE array
is fully used despite c=64 contraction / c=64 output:
  - QK^T: window A occupies PE rows 0-63, window B rows 64-127
    (tile_position row packing; both matmuls run concurrently).
  - attn@V: window A drains to PSUM partitions 0-63, window B to 64-127
    (tile_position column packing).
Operands are bf16 (1 col/cycle streaming + FWL weight loads); accumulation
stays fp32 in PSUM. The relu(PSUM)->SBUF pass is load-balanced between the
Scalar (ACT) and Vector (DVE) engines, which are the throughput floor.
All per-pair inputs ship as ONE merged dram tensor [km | qm | v] so each
pair costs a single input DMA (DMA semaphores are expensive: each burns a
semaphore register that the BSP epilogue later resets one 90ns op at a
time on the Scalar engine).
"""

import sys

sys.path.insert(0, "/opt/trn_rl_repo")

import numpy as np
import ml_dtypes

C = 64          # channels
NW = 64         # windows (8x8 grid of 32x32 patches on 256x256)
T = 1024        # tokens per window (32*32)
NCORES = 8
WPC = NW // NCORES  # windows per core
NPAIR = WPC // 2

_CACHE = {}


def _build_program():
    import concourse.mybir as mybir
    from concourse import bacc
    from concourse.tile import TileContext

    bf16 = mybir.dt.bfloat16
    f32 = mybir.dt.float32

    nc = bacc.Bacc(None, target_bir_lowering=False)
    # merged input, per pair: columns [0:1024] = km, [1024:2048] = qm,
    # [2048:3072] = v as [window-in-pair(2), s-chunk(8), c(64)].
    # km/qm partitions = (window-in-pair, c): window A channels at
    # partitions 0-63, window B at 64-127. v partitions = s-in-chunk.
    in_d = nc.declare_dram_parameter("inp", [NPAIR, 128, 3 * T], bf16, isOutput=False)
    # o: [pair, 128, T]; partitions 0-63 = window A [c, T], 64-127 = window B
    o_d = nc.declare_dram_parameter("o", [NPAIR, 128, T], bf16, isOutput=True)

    # greedy ACT/DVE load balancing for the PSUM->SBUF relu/copy passes
    eng_time = {"act": 0.0, "dve": 0.0}

    def pick_engine():
        if eng_time["act"] <= eng_time["dve"]:
            eng_time["act"] += 1040.0  # ~[128,1024] ACTIVATE ns
            return "act"
        eng_time["dve"] += 1190.0  # ~[128,1024] DVE tensor_scalar ns
        return "dve"

    def relu_to(engine, out_t, in_t):
        if engine == "act":
            nc.scalar.activation(
                out=out_t, in_=in_t,
                func=mybir.ActivationFunctionType.Relu, scale=1.0,
            )
        else:
            nc.vector.tensor_scalar_max(out_t, in_t, 0.0)

    def copy_to(engine, out_t, in_t):
        if engine == "act":
            nc.scalar.activation(
                out=out_t, in_=in_t,
                func=mybir.ActivationFunctionType.Copy, scale=1.0,
            )
        else:
            nc.vector.tensor_copy(out=out_t, in_=in_t)

    with TileContext(nc) as tc:
        with (
            tc.tile_pool(name="qk", bufs=2) as qk_pool,
            tc.tile_pool(name="at", bufs=1) as a_pool,
            tc.tile_pool(name="ob", bufs=2) as o_pool,
            tc.tile_pool(name="wm", bufs=1) as w_pool,
            # 3 rotating [128,T] QK-output tiles (6 banks) + 1 ps_o (2 banks).
            # Rotation is forced with DISTINCT tags (ps0/ps1/ps2) — same-tag
            # tiles were observed to reuse one slot, serializing the pipeline.
            tc.tile_pool(name="pa", bufs=1, space="PSUM") as pa_pool,
            tc.tile_pool(name="po", bufs=1, space="PSUM") as po_pool,
        ):
            # preload the ACT Relu table while the first DMAs are in flight
            warm_t = w_pool.tile([1, 2], f32, tag="warm")
            nc.vector.memset(warm_t, 0.0)
            nc.scalar.activation(
                out=warm_t, in_=warm_t,
                func=mybir.ActivationFunctionType.Relu, scale=1.0,
            )

            for p in range(NPAIR):
                # merged column layout (so the cold-start chunk is small):
                # [0:128] km k=0 | [128:640] qm h0 | [640:1536] km k=1..7 |
                # [1536:2048] qm h1 | [2048:3072] v
                in_t = qk_pool.tile([128, 3 * T], bf16, tag="inp")
                if p == 0:
                    # first chunk = just what QK(0)-h0 needs (160KB)
                    nc.sync.dma_start(out=in_t[:, 0:640], in_=in_d[p, :, 0:640])
                    nc.sync.dma_start(
                        out=in_t[:, 640:3 * T], in_=in_d[p, :, 640:3 * T]
                    )
                else:
                    nc.sync.dma_start(out=in_t, in_=in_d[p])

                def km_ap(k):
                    if k == 0:
                        return in_t[:, 0:128]
                    return in_t[:, 640 + (k - 1) * 128:640 + k * 128]

                def qm_ap(h):
                    return in_t[:, 128:640] if h == 0 else in_t[:, 1536:2048]

                def v_ap(w, k):
                    # v for window-in-pair w, s-chunk k: [128, 64] slice
                    off = 2 * T + w * 512 + k * C
                    return in_t[:, off:off + C]

                def qk_pair(k):
                    # QK^T for s-chunk k: two windows row-packed on the PE
                    # array (A on rows 0-63, B on rows 64-127, concurrent)
                    i = p * 16 + 2 * k
                    ps_a = pa_pool.tile([128, T], f32, tag=f"ps{i % 3}")
                    ps_b = pa_pool.tile([128, T], f32, tag=f"ps{(i + 1) % 3}")
                    km_k = km_ap(k)
                    for h in range(2):
                        hs = slice(h * 512, (h + 1) * 512)
                        qm_h = qm_ap(h)
                        nc.tensor.matmul(
                            out=ps_a[:, hs], lhsT=km_k[0:64, :],
                            rhs=qm_h[0:64, :], start=True, stop=True,
                        )
                        nc.tensor.matmul(
                            out=ps_b[:, hs], lhsT=km_k[64:128, :],
                            rhs=qm_h[64:128, :], start=True, stop=True,
                        )
                    return ps_a, ps_b

                ps_o = po_pool.tile([128, T], f32, tag="pso")
                cur = qk_pair(0)
                for k in range(8):
                    ps_a, ps_b = cur
                    at_a = a_pool.tile([128, T], bf16, tag=f"at_a{k % 2}")
                    at_b = a_pool.tile([128, T], bf16, tag=f"at_b{k % 2}")
                    relu_to(pick_engine(), at_a, ps_a)
                    relu_to(pick_engine(), at_b, ps_b)
                    # keep the PE stream dense: QK(k+1) issues before AV(k)
                    if k < 7:
                        cur = qk_pair(k + 1)
                    # attn @ V: two windows column-packed (A -> psum partitions
                    # 0-63, B -> 64-127), accumulating over s-chunks k
                    for h in range(2):
                        hs = slice(h * 512, (h + 1) * 512)
                        nc.tensor.matmul(
                            out=ps_o[0:64, hs], lhsT=v_ap(0, k),
                            rhs=at_a[:, hs], start=(k == 0), stop=(k == 7),
                        )
                        nc.tensor.matmul(
                            out=ps_o[64:128, hs], lhsT=v_ap(1, k),
                            rhs=at_b[:, hs], start=(k == 0), stop=(k == 7),
                        )
                o_t = o_pool.tile([128, T], bf16, tag="o")
                copy_to(pick_engine(), o_t, ps_o)
                nc.sync.dma_start(out=o_d[p], in_=o_t)

    nc.finalize()
    return nc


def kernel(x, W, bias):
    from concourse.bass_utils import run_bass_kernel_spmd

    x = np.asarray(x, dtype=np.float32)
    W = np.asarray(W, dtype=np.float32)
    bias = np.asarray(bias, dtype=np.float32)

    # ---- host prep: windows, qkv, routing, mixing (tiny vs attention) ----
    # xw: [nw, T, c]
    xw = (
        x.reshape(C, 8, 32, 8, 32)
        .transpose(1, 3, 2, 4, 0)
        .reshape(NW, T, C)
    )
    qkv = xw @ W.T + bias  # [nw, T, 3c]
    q, k, v = qkv[..., :C], qkv[..., C:2 * C], qkv[..., 2 * C:]
    q_r = q.mean(axis=1)  # [nw, c]
    k_r = k.mean(axis=1)
    a_r = np.maximum(q_r @ k_r.T, 0.0)  # [nw, nw]
    k_m = np.tensordot(a_r, k, axes=(1, 0))  # [nw, T, c]
    q_m = np.tensordot(a_r, q, axes=(1, 0))

    if "nc" not in _CACHE:
        _CACHE["nc"] = _build_program()
    nc = _CACHE["nc"]

    bf16 = ml_dtypes.bfloat16
    in_maps = []
    for m in range(NCORES):
        s = slice(m * WPC, (m + 1) * WPC)
        # [wpc, T, c] -> [pair, 2, c, T] -> [pair, 128, T]
        qm_p = (
            q_m[s].reshape(NPAIR, 2, T, C).transpose(0, 1, 3, 2)
            .reshape(NPAIR, 128, T)
        )
        km_p = (
            k_m[s].reshape(NPAIR, 2, T, C).transpose(0, 1, 3, 2)
            .reshape(NPAIR, 128, T)
        )
        # [wpc, T, c] -> [pair, 2, 8, 128, c] -> [pair, 128(s), 2, 8, c]
        v_p = (
            v[s].reshape(NPAIR, 2, 8, 128, C).transpose(0, 3, 1, 2, 4)
            .reshape(NPAIR, 128, T)
        )
        merged = np.concatenate(
            [km_p[:, :, 0:128], qm_p[:, :, 0:512], km_p[:, :, 128:T],
             qm_p[:, :, 512:T], v_p],
            axis=2,
        )  # [pair, 128, 3T]
        in_maps.append({"inp": np.ascontiguousarray(merged).astype(bf16)})

    res = run_bass_kernel_spmd(nc, in_maps, list(range(NCORES)))
    _CACHE["last_res"] = res
    # [pair, 128, T] -> [wpc, c, T] per core
    outs = [
        np.asarray(res.results[m]["o"]).astype(np.float32).reshape(NPAIR * 2, C, T)
        for m in range(NCORES)
    ]
    o_all = np.concatenate(outs, axis=0)  # [nw, c, T]
    o_cm = o_all.transpose(1, 0, 2)  # [c, nw, T]

    # fold back: [c, jh, jw, th, tw] -> [1, c, 256, 256]
    o_img = (
        o_cm.reshape(C, 8, 8, 32, 32)
        .transpose(0, 1, 3, 2, 4)
        .reshape(1, C, 256, 256)
    )
    return o_img.astype(np.float32)


# revision 28
# speedup vs baseline: 1.2794x; 1.2794x over previous
"""Window-routed sparse attention on 8 TRN2 NeuronCores.

Sharding: 64 windows x 8 cores = 8 windows/core (embarrassingly parallel).
Host precomputes the tiny routing path (region means, a_r [64,64]) and the
window-mixed q_m/k_m in fp32 numpy; each core runs the heavy windowed
attention relu(q_m k_m^T) v for its 8 windows.

Device-side layout: windows are processed in PAIRS so the 128x128 PE array
is fully used despite c=64 contraction / c=64 output:
  - QK^T: window A occupies PE rows 0-63, window B rows 64-127
    (tile_position row packing; both matmuls run concurrently).
  - attn@V: window A drains to PSUM partitions 0-63, window B to 64-127
    (tile_position column packing).
Operands are bf16 (1 col/cycle streaming + FWL weight loads); accumulation
stays fp32 in PSUM. The relu(PSUM)->SBUF pass is load-balanced between the
Scalar (ACT) and Vector (DVE) engines, which are the throughput floor.
All per-pair inputs ship as ONE merged dram tensor [km | qm | v] so each
pair costs a single input DMA (DMA semaphores are expensive: each burns a
semaphore register that the BSP epilogue later resets one 90ns op at a
time on the Scalar engine).
"""

import sys

sys.path.insert(0, "/opt/trn_rl_repo")

import numpy as np
import ml_dtypes

C = 64          # channels
NW = 64         # windows (8x8 grid of 32x32 patches on 256x256)
T = 1024        # tokens per window (32*32)
NCORES = 8
WPC = NW // NCORES  # windows per core
NPAIR = WPC // 2

_CACHE = {}


def _build_program():
    import concourse.mybir as mybir
    from concourse import bacc
    from concourse.tile import TileContext

    bf16 = mybir.dt.bfloat16
    f32 = mybir.dt.float32

    nc = bacc.Bacc(None, target_bir_lowering=False)
    # merged input, per pair: columns [0:1024] = km, [1024:2048] = qm,
    # [2048:3072] = v as [window-in-pair(2), s-chunk(8), c(64)].
    # km/qm partitions = (window-in-pair, c): window A channels at
    # partitions 0-63, window B at 64-127. v partitions = s-in-chunk.
    in_d = nc.declare_dram_parameter("inp", [NPAIR, 128, 3 * T], bf16, isOutput=False)
    # o: [pair, 128, T]; partitions 0-63 = window A [c, T], 64-127 = window B
    o_d = nc.declare_dram_parameter("o", [NPAIR, 128, T], bf16, isOutput=True)

    # greedy ACT/DVE load balancing for the PSUM->SBUF relu/copy passes.
    # Weights are MEASURED per-op engine periods (duration + write-ack +
    # semaphore + dispatch), not cost-model durations.
    eng_time = {"act": 0.0, "dve": 0.0}

    def pick_engine(cols=1024):
        act_cost = cols * 1.04 + 310.0
        dve_cost = cols * 1.27 + 290.0
        if eng_time["act"] + act_cost / 2 <= eng_time["dve"] + dve_cost / 2:
            eng_time["act"] += act_cost
            return "act"
        eng_time["dve"] += dve_cost
        return "dve"

    def relu_to(engine, out_t, in_t):
        if engine == "act":
            nc.scalar.activation(
                out=out_t, in_=in_t,
                func=mybir.ActivationFunctionType.Relu, scale=1.0,
            )
        else:
            nc.vector.tensor_scalar_max(out_t, in_t, 0.0)

    def copy_to(engine, out_t, in_t):
        if engine == "act":
            nc.scalar.activation(
                out=out_t, in_=in_t,
                func=mybir.ActivationFunctionType.Copy, scale=1.0,
            )
        else:
            nc.vector.tensor_copy(out=out_t, in_=in_t)

    with TileContext(nc) as tc:
        with (
            tc.tile_pool(name="qk", bufs=2) as qk_pool,
            tc.tile_pool(name="at", bufs=1) as a_pool,
            tc.tile_pool(name="ob", bufs=2) as o_pool,
            tc.tile_pool(name="wm", bufs=1) as w_pool,
            # 3 rotating [128,T] QK-output tiles (6 banks) + 1 ps_o (2 banks).
            # Rotation is forced with DISTINCT tags (ps0/ps1/ps2) — same-tag
            # tiles were observed to reuse one slot, serializing the pipeline.
            tc.tile_pool(name="pa", bufs=1, space="PSUM") as pa_pool,
            tc.tile_pool(name="po", bufs=1, space="PSUM") as po_pool,
        ):
            # preload the ACT Relu table while the first DMAs are in flight
            warm_t = w_pool.tile([1, 2], f32, tag="warm")
            nc.vector.memset(warm_t, 0.0)
            nc.scalar.activation(
                out=warm_t, in_=warm_t,
                func=mybir.ActivationFunctionType.Relu, scale=1.0,
            )

            for p in range(NPAIR):
                # merged column layout (so the cold-start chunk is small):
                # [0:128] km k=0 | [128:640] qm h0 | [640:1536] km k=1..7 |
                # [1536:2048] qm h1 | [2048:3072] v
                in_t = qk_pool.tile([128, 3 * T], bf16, tag="inp")
                if p == 0:
                    # first chunk = just what QK(0)-h0 needs (160KB)
                    nc.sync.dma_start(out=in_t[:, 0:640], in_=in_d[p, :, 0:640])
                    nc.sync.dma_start(
                        out=in_t[:, 640:3 * T], in_=in_d[p, :, 640:3 * T]
                    )
                else:
                    nc.sync.dma_start(out=in_t, in_=in_d[p])

                def km_ap(k):
                    if k == 0:
                        return in_t[:, 0:128]
                    return in_t[:, 640 + (k - 1) * 128:640 + k * 128]

                def qm_ap(h):
                    return in_t[:, 128:640] if h == 0 else in_t[:, 1536:2048]

                def v_ap(w, k):
                    # v for window-in-pair w, s-chunk k: [128, 64] slice
                    off = 2 * T + w * 512 + k * C
                    return in_t[:, off:off + C]

                def qk_pair(k):
                    # QK^T for s-chunk k: one rotating [128,1024] PSUM tile
                    # per q-half holding [window A | window B] side by side.
                    # The two MMs per tile are row-packed (A on PE rows 0-63,
                    # B on 64-127, concurrent) and drain to different banks.
                    # One relu per tile then covers a whole AV column-pair,
                    # and needs only the FIRST QK pair-slot of the cycle.
                    i = p * 16 + 2 * k
                    ps_h0 = pa_pool.tile(
                        [128, T], f32, tag=f"ps{i % 3}", name=f"psh0_{p}_{k}"
                    )
                    ps_h1 = pa_pool.tile(
                        [128, T], f32, tag=f"ps{(i + 1) % 3}",
                        name=f"psh1_{p}_{k}",
                    )
                    for h, ps_h in ((0, ps_h0), (1, ps_h1)):
                        qm_h = qm_ap(h)
                        km_k = km_ap(k)
                        for w in range(2):
                            nc.tensor.matmul(
                                out=ps_h[:, w * 512:(w + 1) * 512],
                                lhsT=km_k[w * 64:(w + 1) * 64, :],
                                rhs=qm_h[w * 64:(w + 1) * 64, :],
                                start=True, stop=True,
                            )
                    return ps_h0, ps_h1

                ps_o = po_pool.tile([128, T], f32, tag="pso")
                cur = qk_pair(0)
                for k in range(8):
                    ps_h0, ps_h1 = cur
                    at_h0 = a_pool.tile(
                        [128, T], bf16, tag=f"at{(2 * k) % 3}",
                        name=f"at0_{p}_{k}",
                    )
                    at_h1 = a_pool.tile(
                        [128, T], bf16, tag=f"at{(2 * k + 1) % 3}",
                        name=f"at1_{p}_{k}",
                    )
                    relu_to(pick_engine(), at_h0, ps_h0)
                    relu_to(pick_engine(), at_h1, ps_h1)
                    # keep the PE stream dense: QK(k+1) issues before AV(k)
                    if k < 7:
                        cur = qk_pair(k + 1)
                    # attn @ V: two windows column-packed (A -> psum partitions
                    # 0-63, B -> 64-127), accumulating over s-chunks k
                    for h, at_h in ((0, at_h0), (1, at_h1)):
                        for w in range(2):
                            nc.tensor.matmul(
                                out=ps_o[w * 64:(w + 1) * 64,
                                         h * 512:(h + 1) * 512],
                                lhsT=v_ap(w, k),
                                rhs=at_h[:, w * 512:(w + 1) * 512],
                                start=(k == 0), stop=(k == 7),
                            )
                o_t = o_pool.tile([128, T], bf16, tag="o")
                copy_to(pick_engine(), o_t, ps_o)
                nc.sync.dma_start(out=o_d[p], in_=o_t)

    nc.finalize()
    return nc


def kernel(x, W, bias):
    from concourse.bass_utils import run_bass_kernel_spmd

    x = np.asarray(x, dtype=np.float32)
    W = np.asarray(W, dtype=np.float32)
    bias = np.asarray(bias, dtype=np.float32)

    # ---- host prep: windows, qkv, routing, mixing (tiny vs attention) ----
    # xw: [nw, T, c]
    xw = (
        x.reshape(C, 8, 32, 8, 32)
        .transpose(1, 3, 2, 4, 0)
        .reshape(NW, T, C)
    )
    qkv = xw @ W.T + bias  # [nw, T, 3c]
    q, k, v = qkv[..., :C], qkv[..., C:2 * C], qkv[..., 2 * C:]
    q_r = q.mean(axis=1)  # [nw, c]
    k_r = k.mean(axis=1)
    a_r = np.maximum(q_r @ k_r.T, 0.0)  # [nw, nw]
    k_m = np.tensordot(a_r, k, axes=(1, 0))  # [nw, T, c]
    q_m = np.tensordot(a_r, q, axes=(1, 0))

    if "nc" not in _CACHE:
        _CACHE["nc"] = _build_program()
    nc = _CACHE["nc"]

    bf16 = ml_dtypes.bfloat16
    in_maps = []
    for m in range(NCORES):
        s = slice(m * WPC, (m + 1) * WPC)
        # [wpc, T, c] -> [pair, 2, c, T] -> [pair, 128, T]
        qm_p = (
            q_m[s].reshape(NPAIR, 2, T, C).transpose(0, 1, 3, 2)
            .reshape(NPAIR, 128, T)
        )
        km_p = (
            k_m[s].reshape(NPAIR, 2, T, C).transpose(0, 1, 3, 2)
            .reshape(NPAIR, 128, T)
        )
        # [wpc, T, c] -> [pair, 2, 8, 128, c] -> [pair, 128(s), 2, 8, c]
        v_p = (
            v[s].reshape(NPAIR, 2, 8, 128, C).transpose(0, 3, 1, 2, 4)
            .reshape(NPAIR, 128, T)
        )
        merged = np.concatenate(
            [km_p[:, :, 0:128], qm_p[:, :, 0:512], km_p[:, :, 128:T],
             qm_p[:, :, 512:T], v_p],
            axis=2,
        )  # [pair, 128, 3T]
        in_maps.append({"inp": np.ascontiguousarray(merged).astype(bf16)})

    res = run_bass_kernel_spmd(nc, in_maps, list(range(NCORES)))
    _CACHE["last_res"] = res
    # [pair, 128, T] -> [wpc, c, T] per core
    outs = [
        np.asarray(res.results[m]["o"]).astype(np.float32).reshape(NPAIR * 2, C, T)
        for m in range(NCORES)
    ]
    o_all = np.concatenate(outs, axis=0)  # [nw, c, T]
    o_cm = o_all.transpose(1, 0, 2)  # [c, nw, T]

    # fold back: [c, jh, jw, th, tw] -> [1, c, 256, 256]
    o_img = (
        o_cm.reshape(C, 8, 8, 32, 32)
        .transpose(0, 1, 3, 2, 4)
        .reshape(1, C, 256, 256)
    )
    return o_img.astype(np.float32)
